# revision 1
# baseline (speedup 1.0000x reference)
"""Trainium2 Bass kernel for EntropyConstrainedRefinement.

Reference semantics (see problem):
  p = frank_wolfe(quality_scores)   # entropy-constrained softmax + FW refine
  out = spatial_coherence(p, masks) # TV-regularized reweighting

Key algebraic facts used:
  * The FW line-search objective is linear in gamma, so gamma is always 0 or 1
    and the 50-iteration loop collapses to:
        p = s  if sum((s - p0) * q) >= 1e-4  else  p0
    where s = softmax(q / lambda*), p0 = uniform(1/K).
  * The 20-step binary search on lambda is equivalent (for the monotone
    entropy-vs-lambda curve) to 4 rounds of locating the entropy crossing on a
    32-point grid of the current interval: new_lmin = lmin + m*w with
    m = #{grid points with ent < 0.6}, w = (lmax - lmin)/32.
  * tv_norm = (tv - min)/(max - min + eps) is scale-invariant (eps
    negligible), so the 1/(H*W) factor is dropped and tv is accumulated
    unscaled.
  * TV is computed in bf16 (masks are converted to bf16 on the host,
    halving HBM traffic; tv_norm tolerance is ~0.18):
      sh    = m shifted left by one element   (ScalarE copy, ~2 elem/cyc)
      dh    = sh - m   over [0:4095]          (DVE tensor_tensor, 2x_1p bf16)
      dv    = m[64:] - m[:4032]               (DVE tensor_tensor, 2x_1p bf16)
      a_h   = sum |dh|                        (ScalarE Abs + accum)
      a_v   = sum |dv|                        (ScalarE Abs + accum)
      a_w   = sum |dh| at w=63 (h<63)         (ScalarE Abs on strided view)
      tv    = a_h + a_v - a_w    (a_w removes the row-wrap h-diff terms)
    bf16 rounding error on tv is ~1e-4 relative; tv_norm tolerance is ~0.18.

Sharding: data-parallel over the batch dim: 16 batches -> 8 cores x 2.
"""

import operator
import os
import sys

import numpy as np

if os.path.isdir("/opt/trn_rl_repo"):
    sys.path.insert(0, "/opt/trn_rl_repo")

import concourse.bacc as bacc
import concourse.bass as bass
import concourse.mybir as mybir
import concourse.tile as tile
from concourse import dve_ops
from concourse.bass_utils import run_bass_kernel_spmd
from concourse.dve_spec import C0, C1, Leaf, Spec, Src0, Src1, lower, maxx
from concourse.dve_uop import DveOpSpec, InpSel

F32 = mybir.dt.float32
BF16 = mybir.dt.bfloat16
AX = mybir.AxisListType
OP = mybir.AluOpType
AF = mybir.ActivationFunctionType

B_PER_CORE = 2
N_CORES = 8
K = 1000
H = W = 64
HW = H * W
KT = 8          # k-interleave factor: k = 8*p + g, p in [0,125), g in [0,8)
KP = K // KT    # 125 partitions per mask tile

ENTROPY_THRESHOLD = 0.6
CONVERGENCE_TOL = 1e-4
LAMBDA_MIN = 0.001
LAMBDA_MAX = 100.0
LAMBDA_REG = 0.1
EPS = 1e-10
P0 = float(np.float32(1.0) / np.float32(K))          # uniform init prob
GRID = 32       # candidates per binary-search round (5 bits); 4 rounds = 20 steps
ROUNDS = 4

# ---------------------------------------------------------------------------
# Custom DVE ops for the TV accumulation.
#
# ABS_DIFF_P2_ANT: accum = s0 + sum(|in0 - in1|). The REGULAR (1x) program
# computes |Src0 - Src1| one element/cycle. A hand-registered 2X_1PORT
# program additionally consumes the packed high bf16 halves
# (SRC_0_HI/SRC_1_HI crossbar lanes) so each cycle processes one 32-bit
# read = TWO bf16 element pairs: |lo0-lo1| + |hi0-hi1|. Both programs
# accumulate the same total; the per-element `out` stream (a dump tile)
# differs by mode and is never read. perf_max=1 on the instruction makes
# the hardware engage the 2x slot when dtype/stride/alignment qualify.
#
# ABS_DIFF_SC_ANT: accum = s0 + sum(|in0 - in1| * s1); 1x only. Used with
# s1=+1/-1 for the odd straggler element and the row-wrap correction.
# ---------------------------------------------------------------------------
_DVE_OPS = None


def _absdiff_ref(in0, in1, c0, c1, c2):
    b = (np.abs(in0.astype(np.float32) - in1.astype(np.float32)) * np.float32(c1)).astype(
        np.float32
    )
    return b, c0 + b.reshape(b.shape[0], -1).sum(axis=-1, keepdims=True)


def _absdiff_noscale_ref(in0, in1, c0, c1, c2):
    b = np.abs(in0.astype(np.float32) - in1.astype(np.float32)).astype(np.float32)
    return b, c0 + b.reshape(b.shape[0], -1).sum(axis=-1, keepdims=True)


def _get_dve_ops():
    global _DVE_OPS
    if _DVE_OPS is not None:
        return _DVE_OPS
    by_name = {o.name: o for o in dve_ops.OPS}
    src0hi, src1hi = Leaf(InpSel.SRC_0_HI), Leaf(InpSel.SRC_1_HI)
    ver = "v3"  # TRN2 (dve_ver_for)

    def register(name, spec, spec2x=None):
        if name in by_name:
            return by_name[name]
        row = dve_ops._CUSTOM_DVE_ROW_BASE + len(dve_ops.OPS)
        assert row < 0x20, "custom DVE opcode row overflow"
        dve_ops._SUB_OPCODE_FOR_NAME[name] = row
        dspec = DveOpSpec(
            name=name,
            opcode=row,
            uops=lower(spec, ver=ver),
            uops_2x=lower(spec2x, ver=ver) if spec2x is not None else None,
            rd1_en=True,
            perf_max=1 if spec2x is not None else 0,
        )
        dspec.validate(ver)
        op = dve_ops.DveOp(name, spec, subdim=False, uops_sha={ver: dspec.sha(ver)})
        # Pre-seed the compile cache so DveOp.compile(ver) returns our spec
        # (with the 2x slot) instead of re-lowering just the 1x body.
        dve_ops._COMPILE_CACHE[(name, ver)] = dspec
        dve_ops.OPS.append(op)
        return op

    p2 = register(
        "ABS_DIFF_P2_ANT",
        Spec(
            body=maxx(Src0 - Src1, Src1 - Src0),
            accum=operator.add, accum_init=C0, reference=_absdiff_noscale_ref,
        ),
        Spec(
            body=maxx(Src0 - Src1, Src1 - Src0)
            + maxx(src0hi - src1hi, src1hi - src0hi),
            accum=operator.add, accum_init=C0, reference=_absdiff_noscale_ref,
        ),
    )
    sc = register(
        "ABS_DIFF_SC_ANT",
        Spec(
            body=maxx(Src0 - Src1, Src1 - Src0) * C1,
            accum=operator.add, accum_init=C0, reference=_absdiff_ref,
        ),
    )
    _DVE_OPS = (p2, sc)
    return _DVE_OPS


# ---------------------------------------------------------------------------
# Host-side constants shipped as inputs
# ---------------------------------------------------------------------------
def _make_consts():
    p = np.arange(2 * GRID)
    sel = np.zeros((B_PER_CORE, 2 * GRID), np.float32)      # lhsT: [2,1]-> [64,1] bcast
    sel[p // GRID, p] = 1.0
    bcols = sel.T.copy()                                     # [64,2] batch indicator
    iota = ((p % GRID) + 1).astype(np.float32)[:, None]      # j = 1..32
    ones = np.ones((2 * GRID, 1), np.float32)
    return sel, bcols, iota, ones


# ---------------------------------------------------------------------------
# Bass program (identical on all 8 cores)
# ---------------------------------------------------------------------------
def _emit(tc, nc, q, masks, c_sel, c_bcols, c_iota, c_ones, out, ctx, dbg=None):
    p2op, scop = _get_dve_ops()
    la = int(os.environ.get("KERNEL_LA", "2"))
    consts = ctx.enter_context(tc.tile_pool(name="consts", bufs=1))
    mask_pool = ctx.enter_context(tc.tile_pool(name="maskp", bufs=la + 3))
    sh_pool = ctx.enter_context(tc.tile_pool(name="shp", bufs=la + 1))
    d_pool = ctx.enter_context(
        tc.tile_pool(name="dp", bufs=int(os.environ.get("KERNEL_DB", "3")))
    )
    dump_pool = ctx.enter_context(tc.tile_pool(name="dump", bufs=1))
    stage_pool = ctx.enter_context(tc.tile_pool(name="stage", bufs=1))
    small = ctx.enter_context(tc.tile_pool(name="small", bufs=4))
    accs = ctx.enter_context(tc.tile_pool(name="accs", bufs=4))
    big = ctx.enter_context(tc.tile_pool(name="big", bufs=1))
    fin = ctx.enter_context(tc.tile_pool(name="fin", bufs=1))
    psum = ctx.enter_context(tc.tile_pool(name="psum", bufs=2, space="PSUM"))

    # ---------------- constants / q staging ----------------
    q2 = consts.tile([B_PER_CORE, K], F32)
    nc.sync.dma_start(out=q2, in_=q)
    qB = consts.tile([2 * GRID, K], F32)
    for b in range(B_PER_CORE):
        qb = q[b : b + 1, :]
        bro = bass.AP(tensor=qb.tensor, offset=qb.offset, ap=[[0, GRID]] + qb.ap[1:])
        nc.sync.dma_start(out=qB[b * GRID : (b + 1) * GRID, :], in_=bro)
    selT = consts.tile([B_PER_CORE, 2 * GRID], F32)
    nc.sync.dma_start(out=selT, in_=c_sel)
    bcolsT = consts.tile([2 * GRID, B_PER_CORE], F32)
    nc.sync.dma_start(out=bcolsT, in_=c_bcols)
    iotaT = consts.tile([2 * GRID, 1], F32)
    nc.sync.dma_start(out=iotaT, in_=c_iota)
    onesT = consts.tile([2 * GRID, 1], F32)
    nc.sync.dma_start(out=onesT, in_=c_ones)

    qmaxB = consts.tile([2 * GRID, 1], F32)
    nc.vector.reduce_max(out=qmaxB, in_=qB, axis=AX.X)
    epsb = consts.tile([2 * GRID, 1], F32)
    nc.vector.memset(epsb, EPS)
    qmax2 = consts.tile([B_PER_CORE, 1], F32)
    nc.vector.reduce_max(out=qmax2, in_=q2, axis=AX.X)

    # ---------------- TV over masks: DMA + shift-copy emission ----------------
    # masks[b, k] with k = 8*p + g  ->  masksR[b, g, p, :] rows of 8KB (bf16).
    # Emission is software-pipelined: tile t's DMA + ScalarE shift-copy are
    # emitted LOOKAHEAD tiles before tile t's DVE/ScalarE reduction work, so
    # neither engine queue convoys behind the other.
    masksR = masks.rearrange("b (p g) h w -> b g p (h w)", g=KT)
    dumpA = dump_pool.tile([128, HW], BF16, name="dumpA")
    dumpV = dump_pool.tile([128, HW - H], BF16, name="dumpV")
    dumpW = dump_pool.tile([128, H], F32, name="dumpW")
    dumpW3 = dumpW[:KP, 0 : H - 1].rearrange("p (h w) -> p h w", h=H - 1, w=1)
    hv_st = [stage_pool.tile([128, KT], F32, name=f"hv{b}") for b in range(B_PER_CORE)]
    ah_st = [stage_pool.tile([128, KT], F32, name=f"ah{b}") for b in range(B_PER_CORE)]
    av_st = [stage_pool.tile([128, KT], F32, name=f"av{b}") for b in range(B_PER_CORE)]
    aw_st = [stage_pool.tile([128, KT], F32, name=f"aw{b}") for b in range(B_PER_CORE)]

    n_vsc = int(os.environ.get("KERNEL_VSC", "6"))
    V_SC = frozenset(range(0, 2 * n_vsc, 2)[:n_vsc])
    LOOKAHEAD = la
    tiles = [(b, g) for b in range(B_PER_CORE) for g in range(KT)]
    staged = {}

    def stage_tile(t):
        b, g = tiles[t]
        mt = mask_pool.tile([128, HW], BF16, tag="mask")
        nc.sync.dma_start(out=mt[:KP, :], in_=masksR[b, g])
        # sh[i] = m[i+1], i in [0, 4095)
        sh = sh_pool.tile([128, HW], BF16, tag="sh")
        nc.scalar.activation(out=sh[:KP, 0 : HW - 1], in_=mt[:KP, 1:HW], func=AF.Copy)
        staged[t] = (mt, sh)

    def reduce_tile(t):
        b, g = tiles[t]
        mt, sh = staged.pop(t)
        # dh = sh - m over [0:4095)  (bf16 2x_1p tensor_tensor)
        dh = d_pool.tile([128, HW], BF16, tag="dh")
        nc.vector.tensor_tensor(
            out=dh[:KP, 0 : HW - 1], in0=sh[:KP, 0 : HW - 1],
            in1=mt[:KP, 0 : HW - 1], op=OP.subtract,
        )
        # a_h = sum |dh|  (includes the 63 row-wrap terms)
        nc.scalar.activation(
            out=dumpA[:KP, 0 : HW - 1], in_=dh[:KP, 0 : HW - 1], func=AF.Abs,
            accum_out=ah_st[b][:KP, g : g + 1],
        )
        # a_w = sum_h<63 |dh[64h + 63]|  (row-wrap terms to remove)
        dh3 = dh[:KP, :].rearrange("p (h w) -> p h w", h=H, w=W)
        nc.scalar.activation(
            out=dumpW3, in_=dh3[:, 0 : H - 1, W - 1 : W], func=AF.Abs,
            accum_out=aw_st[b][:KP, g : g + 1],
        )
        if t in V_SC:
            # a_v via the fused 1x custom op (subtract+abs+accum on DVE)
            nc.vector._custom_dve(
                scop, out=dumpV[:KP, :], in0=mt[:KP, H:HW],
                in1=mt[:KP, 0 : HW - H], s0=0.0, s1=1.0,
                accum_out=av_st[b][:KP, g : g + 1],
            )
        else:
            dv = d_pool.tile([128, HW - H], BF16, tag="dv")
            nc.vector.tensor_tensor(
                out=dv[:KP, :], in0=mt[:KP, H:HW], in1=mt[:KP, 0 : HW - H],
                op=OP.subtract,
            )
            nc.scalar.activation(
                out=dumpA[:KP, 0 : HW - H], in_=dv[:KP, :], func=AF.Abs,
                accum_out=av_st[b][:KP, g : g + 1],
            )

    for t in range(min(LOOKAHEAD + 1, len(tiles))):
        stage_tile(t)

    # ---------------- binary search on lambda (4 x 5 bits) ----------------
    # Emitted before the TV reductions: it depends only on q, so its serial
    # dependency chain executes under the mask-DMA ramp.
    lmin = small.tile([B_PER_CORE, 1], F32, tag="lmin")
    nc.vector.memset(lmin, LAMBDA_MIN)
    lmax = small.tile([B_PER_CORE, 1], F32, tag="lmax")
    nc.vector.memset(lmax, LAMBDA_MAX)

    if dbg is not None:
        dbg_ent_st = stage_pool.tile([2 * GRID, ROUNDS], F32, name="dbg_ent_st")
        dbg_cnt_st = stage_pool.tile([B_PER_CORE, ROUNDS], F32, name="dbg_cnt_st")

    for rnd in range(ROUNDS):
        lmw = small.tile([B_PER_CORE, 2], F32, tag="lmw")
        # col0 = lmin, col1 = w = (lmax - lmin)/GRID
        nc.vector.tensor_copy(out=lmw[:, 0:1], in_=lmin)
        nc.vector.tensor_scalar(
            out=lmw[:, 1:2], in0=lmax, scalar1=lmin, scalar2=1.0 / GRID,
            op0=OP.subtract, op1=OP.mult,
        )
        lw = psum.tile([2 * GRID, 2], F32, tag="lw")
        nc.tensor.matmul(lw, selT, lmw, start=True, stop=True)
        lam = small.tile([2 * GRID, 1], F32, tag="lam")
        nc.vector.scalar_tensor_tensor(
            out=lam, in0=iotaT, scalar=lw[:, 1:2], in1=lw[:, 0:1],
            op0=OP.mult, op1=OP.add,
        )
        inv = small.tile([2 * GRID, 1], F32, tag="inv")
        nc.vector.reciprocal(out=inv, in_=lam)
        nbias = small.tile([2 * GRID, 1], F32, tag="nbias")
        nc.vector.scalar_tensor_tensor(
            out=nbias, in0=qmaxB, scalar=-1.0, in1=inv, op0=OP.mult, op1=OP.mult
        )
        ebuf = big.tile([2 * GRID, K], F32, tag="ebuf")
        zsum = small.tile([2 * GRID, 1], F32, tag="zsum")
        # e = exp(q/lam - qmax/lam), Z = sum(e)
        nc.scalar.activation(
            out=ebuf, in_=qB, func=AF.Exp, bias=nbias, scale=inv, accum_out=zsum
        )
        invz = small.tile([2 * GRID, 1], F32, tag="invz")
        nc.vector.reciprocal(out=invz, in_=zsum)
        ps = big.tile([2 * GRID, K], F32, tag="ps")
        s0 = small.tile([2 * GRID, 1], F32, tag="s0")
        # ps = e/Z ; S0 = sum(ps)   (with accum_out, op1 is the REDUCE op)
        nc.vector.tensor_scalar(
            out=ps, in0=ebuf, scalar1=invz, scalar2=None, op0=OP.mult, op1=OP.add,
            accum_out=s0,
        )
        s2 = small.tile([2 * GRID, 1], F32, tag="s2")
        # S2 = sum(ps + EPS) = S0 + K*EPS
        nc.vector.tensor_scalar(
            out=s2, in0=s0, scalar1=float(np.float32(K * EPS)), scalar2=None,
            op0=OP.add,
        )
        lbuf = big.tile([2 * GRID, K], F32, tag="lbuf")
        # ln(p_safe) = ln(ps + EPS) via the activation's affine pre-add
        nc.scalar.activation(out=lbuf, in_=ps, func=AF.Ln, bias=epsb, scale=1.0)
        mbuf = big.tile([2 * GRID, K], F32, tag="mbuf")
        sm = small.tile([2 * GRID, 1], F32, tag="sm")
        # SM = sum((ps + EPS) * ln(ps + EPS))
        nc.vector.scalar_tensor_tensor(
            out=mbuf, in0=ps, scalar=EPS, in1=lbuf, op0=OP.add, op1=OP.mult,
            accum_out=sm,
        )
        lns2 = small.tile([2 * GRID, 1], F32, tag="lns2")
        nc.scalar.activation(out=lns2, in_=s2, func=AF.Ln)
        invs2 = small.tile([2 * GRID, 1], F32, tag="invs2")
        nc.vector.reciprocal(out=invs2, in_=s2)
        negent = small.tile([2 * GRID, 1], F32, tag="negent")
        # -ent = SM/S2 - ln(S2)
        nc.vector.scalar_tensor_tensor(
            out=negent, in0=sm, scalar=invs2, in1=lns2, op0=OP.mult, op1=OP.subtract
        )
        bm = small.tile([2 * GRID, 1], F32, tag="bm")
        nc.vector.tensor_scalar(
            out=bm, in0=negent, scalar1=-ENTROPY_THRESHOLD, scalar2=None, op0=OP.is_gt
        )
        mcols = small.tile([2 * GRID, B_PER_CORE], F32, tag="mcols")
        nc.vector.tensor_scalar(
            out=mcols, in0=bcolsT, scalar1=bm, scalar2=None, op0=OP.mult
        )
        mcnt = psum.tile([B_PER_CORE, 1], F32, tag="mcnt")
        nc.tensor.matmul(mcnt, mcols, onesT, start=True, stop=True)
        lmin_new = small.tile([B_PER_CORE, 1], F32, tag="lmin")
        nc.vector.scalar_tensor_tensor(
            out=lmin_new, in0=mcnt, scalar=lmw[:, 1:2], in1=lmin,
            op0=OP.mult, op1=OP.add,
        )
        lmax_new = small.tile([B_PER_CORE, 1], F32, tag="lmax")
        nc.vector.tensor_tensor(out=lmax_new, in0=lmin_new, in1=lmw[:, 1:2], op=OP.add)
        if dbg is not None:
            nc.vector.tensor_copy(out=dbg_ent_st[:, rnd : rnd + 1], in_=negent)
            nc.vector.tensor_copy(out=dbg_cnt_st[:, rnd : rnd + 1], in_=mcnt)
        lmin, lmax = lmin_new, lmax_new

    # ---------------- s = softmax(q/lmax); FW-collapsed select ----------------
    inv2 = small.tile([B_PER_CORE, 1], F32, tag="inv2")
    nc.vector.reciprocal(out=inv2, in_=lmax)
    nb2 = small.tile([B_PER_CORE, 1], F32, tag="nb2")
    nc.vector.scalar_tensor_tensor(
        out=nb2, in0=qmax2, scalar=-1.0, in1=inv2, op0=OP.mult, op1=OP.mult
    )
    e2 = fin.tile([B_PER_CORE, K], F32, tag="e2")
    z2 = small.tile([B_PER_CORE, 1], F32, tag="z2")
    nc.scalar.activation(
        out=e2, in_=q2, func=AF.Exp, bias=nb2, scale=inv2, accum_out=z2
    )
    invz2 = small.tile([B_PER_CORE, 1], F32, tag="invz2")
    nc.vector.reciprocal(out=invz2, in_=z2)
    s_t = fin.tile([B_PER_CORE, K], F32, tag="s_t")
    nc.vector.tensor_scalar(out=s_t, in0=e2, scalar1=invz2, scalar2=None, op0=OP.mult)
    # improvement = sum((s - p0) * q); take = improvement >= tol
    impb = fin.tile([B_PER_CORE, K], F32, tag="impb")
    imp = small.tile([B_PER_CORE, 1], F32, tag="imp")
    nc.vector.scalar_tensor_tensor(
        out=impb, in0=s_t, scalar=P0, in1=q2, op0=OP.subtract, op1=OP.mult,
        accum_out=imp,
    )
    take = small.tile([B_PER_CORE, 1], F32, tag="take")
    nc.vector.tensor_scalar(
        out=take, in0=imp, scalar1=CONVERGENCE_TOL, scalar2=None, op0=OP.is_ge
    )
    # p = s*take + p0*(1-take)   (exact: products by 1.0/0.0)
    u_t = fin.tile([B_PER_CORE, K], F32, tag="u_t")
    nc.vector.tensor_scalar(out=u_t, in0=s_t, scalar1=take, scalar2=None, op0=OP.mult)
    ntk = small.tile([B_PER_CORE, 1], F32, tag="ntk")
    nc.vector.tensor_scalar(
        out=ntk, in0=take, scalar1=-P0, scalar2=P0, op0=OP.mult, op1=OP.add
    )
    p_t = fin.tile([B_PER_CORE, K], F32, tag="p_t")
    nc.vector.tensor_scalar(out=p_t, in0=u_t, scalar1=ntk, scalar2=None, op0=OP.add)

    # ---------------- TV reductions, pipelined against staging ----------------
    for t in range(len(tiles)):
        nxt = t + LOOKAHEAD + 1
        if nxt < len(tiles):
            stage_tile(nxt)
        reduce_tile(t)
    for b in range(B_PER_CORE):
        # hv = (a_h + a_v) - a_w, batched over the 8 columns per batch row
        nc.vector.tensor_tensor(
            out=hv_st[b][:KP, :], in0=ah_st[b][:KP, :], in1=av_st[b][:KP, :],
            op=OP.add,
        )
        nc.vector.tensor_tensor(
            out=hv_st[b][:KP, :], in0=hv_st[b][:KP, :], in1=aw_st[b][:KP, :],
            op=OP.subtract,
        )

    # ---------------- spatial coherence ----------------
    tvT = fin.tile([B_PER_CORE, K], F32, tag="tvT")
    for b in range(B_PER_CORE):
        # [125 partitions, 8] -> [1, 1000] in k = 8p+g order
        nc.sync.dma_start(out=tvT[b : b + 1, :], in_=hv_st[b][:KP, :])
    tvmin = small.tile([B_PER_CORE, 1], F32, tag="tvmin")
    nc.vector.tensor_reduce(out=tvmin, in_=tvT, axis=AX.X, op=OP.min)
    tvmax = small.tile([B_PER_CORE, 1], F32, tag="tvmax")
    nc.vector.reduce_max(out=tvmax, in_=tvT, axis=AX.X)
    rng = small.tile([B_PER_CORE, 1], F32, tag="rng")
    nc.vector.scalar_tensor_tensor(
        out=rng, in0=tvmax, scalar=EPS, in1=tvmin, op0=OP.add, op1=OP.subtract
    )
    invr = small.tile([B_PER_CORE, 1], F32, tag="invr")
    nc.vector.reciprocal(out=invr, in_=rng)
    s1c = small.tile([B_PER_CORE, 1], F32, tag="s1c")
    nc.vector.tensor_scalar(
        out=s1c, in0=invr, scalar1=-LAMBDA_REG, scalar2=None, op0=OP.mult
    )
    a_t = fin.tile([B_PER_CORE, K], F32, tag="a_t")
    nc.vector.tensor_scalar(
        out=a_t, in0=tvT, scalar1=tvmin, scalar2=None, op0=OP.subtract
    )
    # c = 0.1 - 0.1 * tv_norm  (= LAMBDA_REG * tv_weight)
    c_t = fin.tile([B_PER_CORE, K], F32, tag="c_t")
    nc.vector.tensor_scalar(
        out=c_t, in0=a_t, scalar1=s1c, scalar2=LAMBDA_REG, op0=OP.mult, op1=OP.add
    )
    pr = fin.tile([B_PER_CORE, K], F32, tag="pr")
    s3 = small.tile([B_PER_CORE, 1], F32, tag="s3")
    nc.vector.scalar_tensor_tensor(
        out=pr, in0=p_t, scalar=1.0 - LAMBDA_REG, in1=c_t, op0=OP.mult, op1=OP.add,
        accum_out=s3,
    )
    invs3 = small.tile([B_PER_CORE, 1], F32, tag="invs3")
    nc.vector.reciprocal(out=invs3, in_=s3)
    o_t = fin.tile([B_PER_CORE, K], F32, tag="o_t")
    nc.vector.tensor_scalar(out=o_t, in0=pr, scalar1=invs3, scalar2=None, op0=OP.mult)
    nc.sync.dma_start(out=out, in_=o_t)

    if dbg is not None:
        nc.sync.dma_start(out=dbg["dbg_lmax"], in_=lmax)
        nc.sync.dma_start(out=dbg["dbg_s"], in_=s_t)
        nc.sync.dma_start(out=dbg["dbg_tv"], in_=tvT)
        nc.sync.dma_start(out=dbg["dbg_take"], in_=take)
        nc.sync.dma_start(out=dbg["dbg_ent"], in_=dbg_ent_st)
        nc.sync.dma_start(out=dbg["dbg_cnt"], in_=dbg_cnt_st)


def _build_program(reps=1):
    nc = bacc.Bacc("TRN2", target_bir_lowering=False, debug=False, num_devices=N_CORES)
    q = nc.dram_tensor("q", [B_PER_CORE, K], F32, kind="ExternalInput").ap()
    masks = nc.dram_tensor(
        "masks", [B_PER_CORE, K, H, W], BF16, kind="ExternalInput"
    ).ap()
    c_sel = nc.dram_tensor("c_sel", [B_PER_CORE, 2 * GRID], F32, kind="ExternalInput").ap()
    c_bcols = nc.dram_tensor("c_bcols", [2 * GRID, B_PER_CORE], F32, kind="ExternalInput").ap()
    c_iota = nc.dram_tensor("c_iota", [2 * GRID, 1], F32, kind="ExternalInput").ap()
    c_ones = nc.dram_tensor("c_ones", [2 * GRID, 1], F32, kind="ExternalInput").ap()
    out = nc.dram_tensor("out", [B_PER_CORE, K], F32, kind="ExternalOutput").ap()

    dbg = None
    if int(os.environ.get("KERNEL_DEBUG", "0")):
        dbg = {
            "dbg_lmax": nc.dram_tensor("dbg_lmax", [B_PER_CORE, 1], F32, kind="ExternalOutput").ap(),
            "dbg_s": nc.dram_tensor("dbg_s", [B_PER_CORE, K], F32, kind="ExternalOutput").ap(),
            "dbg_tv": nc.dram_tensor("dbg_tv", [B_PER_CORE, K], F32, kind="ExternalOutput").ap(),
            "dbg_take": nc.dram_tensor("dbg_take", [B_PER_CORE, 1], F32, kind="ExternalOutput").ap(),
            "dbg_ent": nc.dram_tensor("dbg_ent", [2 * GRID, ROUNDS], F32, kind="ExternalOutput").ap(),
            "dbg_cnt": nc.dram_tensor("dbg_cnt", [B_PER_CORE, ROUNDS], F32, kind="ExternalOutput").ap(),
        }

    from contextlib import ExitStack

    with tile.TileContext(nc) as tc:
        for _ in range(reps):
            with ExitStack() as ctx:
                _emit(tc, nc, q, masks, c_sel, c_bcols, c_iota, c_ones, out, ctx, dbg=dbg)
    nc.compile()
    return nc


_NC_CACHE = None
LAST_RESULT = None  # BassKernelResults of the most recent run (for profiling)


def make_in_maps(q, m):
    bf16 = mybir.dt.np(BF16)
    sel, bcols, iota, ones = _make_consts()
    in_maps = []
    for c in range(N_CORES):
        lo = c * B_PER_CORE
        in_maps.append(
            {
                "q": np.ascontiguousarray(q[lo : lo + B_PER_CORE]),
                # masks ship as bf16: halves HBM traffic; the TV pipeline is
                # bf16 on-device anyway (tv_norm tolerance is ~0.18)
                "masks": np.ascontiguousarray(m[lo : lo + B_PER_CORE].astype(bf16)),
                "c_sel": sel,
                "c_bcols": bcols,
                "c_iota": iota,
                "c_ones": ones,
            }
        )
    return in_maps


def kernel(scores, quality_scores, masks):
    global _NC_CACHE, LAST_RESULT
    del scores  # unused by the reference forward pass
    q = np.ascontiguousarray(np.asarray(quality_scores, dtype=np.float32))
    m = np.ascontiguousarray(np.asarray(masks, dtype=np.float32))
    assert q.shape == (N_CORES * B_PER_CORE, K) and m.shape == (
        N_CORES * B_PER_CORE, K, H, W,
    )

    if _NC_CACHE is None:
        _NC_CACHE = _build_program()
    nc = _NC_CACHE

    in_maps = make_in_maps(q, m)

    trace = bool(int(os.environ.get("KERNEL_TRACE", "0")))
    res = run_bass_kernel_spmd(nc, in_maps, core_ids=list(range(N_CORES)), trace=trace)
    LAST_RESULT = res
    return np.concatenate([r["out"] for r in res.results], axis=0)

